# revision 24
# baseline (speedup 1.0000x reference)
"""Correlation cost-volume kernel for Trainium2 (8 NeuronCores, data-parallel over batch).

cost[b, d, h, w] = mean_c left[b, c, h, w] * right[b, c, h, w - d]   (0 for w < d)

Per (b, h) this is the 48-wide band of the Gram matrix G = L^T R (K = c = 128).
Pipeline per (h-group of HC, w-block):
  bf16 Gram matmuls (PE) -> PSUM (4 h per 2-bank tile)
  -> scaled bf16 copy to SBUF X[i, f*HC + hh] (ACT/DVE alternate, h-interleaved)
  -> shear DMA: S[i, dd*HC + hh] = X[i, (i+dd)*HC + hh]  (flat fused-step AP;
     partition chunks avoid the HW DGE bug: counts not in {64,128}, offset 0)
  -> PE transposes of S 96-col slices -> PSUM [96, m] -> copy to O
  -> DMA to out[d, h, w] (one DMA per dd_l, negative d-stride in dim1).

Queue plan: SWDGE queue 0 carries only the big casting input loads; the
bf16 shears ride the sync HWDGE ring, out-DMAs the scalar HWDGE ring.
"""

import sys
from contextlib import ExitStack

import numpy as np

if "/opt/trn_rl_repo" not in sys.path:
    sys.path.insert(0, "/opt/trn_rl_repo")

import concourse.bass as bass
import concourse.mybir as mybir
from concourse import bacc, tile
from concourse.ap import AP

B = 8
C = 128
H = 160
W = 320
D = 48
PAD = D - 1  # 47
HC = 32  # h rows per group
FW = PAD + 128  # 175, X f-slots per h row
MDT = mybir.dt.bfloat16  # matmul input dtype
SDT = mybir.dt.bfloat16  # S (post-mean) dtype

# w-blocks: (wb, M)
WBLOCKS = [(0, 128), (128, 128), (256, 64)]


def _ncols(wb, m):
    w0 = max(0, wb - PAD)
    return min(W, wb + m) - w0, w0


def build_nc(h=H):
    nc = bacc.Bacc("TRN2", target_bir_lowering=False, debug=False)
    left_d = nc.dram_tensor("left", [C, h, W], mybir.dt.float32, kind="ExternalInput")
    right_d = nc.dram_tensor("right", [C, h, W], mybir.dt.float32, kind="ExternalInput")
    ident_d = nc.dram_tensor("ident", [128, 128], mybir.dt.float32, kind="ExternalInput")
    out_d = nc.dram_tensor("out", [D, h, W], mybir.dt.float32, kind="ExternalOutput")

    ngroups = h // HC
    hw = h * W

    with tile.TileContext(nc) as tc, ExitStack() as ctx:
        const_pool = ctx.enter_context(tc.tile_pool(name="const", bufs=1))
        lr_pool = ctx.enter_context(tc.tile_pool(name="lr", bufs=2))
        x_pool = ctx.enter_context(tc.tile_pool(name="x", bufs=2))
        s_pool = ctx.enter_context(tc.tile_pool(name="s", bufs=2))
        o_pool = ctx.enter_context(tc.tile_pool(name="o", bufs=2))
        g_pool = ctx.enter_context(tc.tile_pool(name="g", bufs=2, space="PSUM"))
        t_pool = ctx.enter_context(tc.tile_pool(name="t", bufs=3, space="PSUM"))

        ident = const_pool.tile([128, 128], SDT)
        nc.gpsimd.dma_start(ident[:], ident_d[:])

        copy_parity = 0

        HH = HC // 2  # 16 h rows per half-load
        for g in range(ngroups):
            h0 = g * HC
            # Half-group loads: halves the startup bubble before the first
            # matmul and lets the load queue refill at finer grain.
            lts, rts = [], []
            for half in range(2):
                lt = lr_pool.tile([C, HH * W], MDT, tag=f"L{half}")
                rt = lr_pool.tile([C, HH * W], MDT, tag=f"R{half}")
                hs = h0 + half * HH
                # SWDGE DMA with fp32 -> bf16 cast
                nc.gpsimd.dma_start(
                    lt[:].rearrange("p (a b) -> p a b", a=HH), left_d[:, hs : hs + HH, :]
                )
                nc.gpsimd.dma_start(
                    rt[:].rearrange("p (a b) -> p a b", a=HH), right_d[:, hs : hs + HH, :]
                )
                lts.append(lt)
                rts.append(rt)

            # O split by w region: A covers w<256 (wblocks 0+128) so its
            # out-DMA can issue before the last wblock computes; B covers
            # the w>=256 tail (1024B / 256B dst runs respectively).
            otileA = o_pool.tile([3 * HC, 16 * 256], mybir.dt.float32, tag="OA")
            otileB = o_pool.tile([3 * HC, 16 * 64], mybir.dt.float32, tag="OB")

            for wb, m in WBLOCKS:
                ncols, w0 = _ncols(wb, m)
                foff = PAD - wb + w0  # 47 for wb=0 else 0
                # X is h-interleaved: X[i, f*HC + hh] = G_hh[i, f]. Two tiles:
                # PSUM copies land interleaved in fp32 X32 (strided 2B writes
                # are slow and strided PSUM reads wedge the device), then the
                # Pool engine does a contiguous fp32->bf16 cast into X so the
                # shear needs no cast and can run on the HWDGE rings.
                x32 = x_pool.tile([m, HC * FW], mybir.dt.float32, tag="X32")
                xtile = x_pool.tile([m, HC * FW], SDT, tag="X")

                if foff:
                    # zero-fill f < 47 region (outputs with w < d)
                    nc.gpsimd.memset(xtile[:, : foff * HC], 0.0)

                for hq in range(HC // 4):
                    # 4 matmuls per 2-bank PSUM tile at 256-slot alignment
                    gt = g_pool.tile([m, 1024], mybir.dt.float32, tag="G")
                    for s in range(4):
                        hh = 4 * hq + s
                        half, hh2 = divmod(hh, HH)
                        nc.tensor.matmul(
                            gt[:, s * 256 : s * 256 + ncols],
                            lts[half][:, hh2 * W + wb : hh2 * W + wb + m],
                            rts[half][:, hh2 * W + w0 : hh2 * W + w0 + ncols],
                            start=True,
                            stop=True,
                        )
                    # PSUM -> X32 with 1/C scaling; dst interleaved (f stride HC)
                    dst = AP(
                        x32[:].tensor,
                        x32[:].offset + foff * HC + 4 * hq,
                        [[HC * FW, m], [1, 4], [HC, ncols]],
                    )
                    src = AP(gt[:].tensor, gt[:].offset, [[1024, m], [256, 4], [1, ncols]])
                    if copy_parity & 1:
                        nc.scalar.mul(dst, src, 1.0 / C)
                    else:
                        nc.vector.tensor_scalar_mul(dst, src, 1.0 / C)
                    copy_parity += 1

                # contiguous casts of the read column ranges into bf16
                # (NOT on gpsimd: Pool tensor ops run at ~11 G el/s; NOT
                # partition-split: partition-offset dst crashes walrus).
                # Low partitions read cols [c0, c1lo) of X; the high-chunk
                # window is cast straight into the partition-shifted scratch
                # xhi, replacing the old rectangular staging DMA hop. The
                # low cast is column-split across both engines for latency.
                p_lo = 96 if m == 128 else 56
                cnt = m - p_lo
                l2 = (cnt - 1) * HC + HC * D
                c0 = foff * HC
                c1 = (foff + ncols) * HC
                cmid = ((c0 + c1) // (2 * HC)) * HC

                def _cp(use_scalar, dst, src):
                    if use_scalar:
                        nc.scalar.copy(dst, src)
                    else:
                        nc.vector.tensor_copy(dst, src)

                par = copy_parity & 1
                _cp(par, xtile[:, c0:cmid], x32[:, c0:cmid])
                _cp(1 - par, xtile[:, cmid:c1], x32[:, cmid:c1])
                copy_parity += 1

                # shear: S[i, dd*HC + hh] = X[i, (i+dd)*HC + hh]; per-partition
                # window is one contiguous HC*D run -> 2-dim flat AP with a
                # fused (row+byte) step. HW DGE constraints: fused-step APs
                # are only correct with offset < row width and partition
                # count not in {64, 128}. Low chunk direct from offset 0;
                # high chunk partition-shifted to rows [0,cnt) of a scratch
                # tile (rectangular DMA), then fused-read at offset 0.
                stile = s_pool.tile([m, HC * D], SDT, tag="S")
                # HWDGE (no cast needed, X/xhi are bf16): keeps SWDGE queue
                # 0 dedicated to the input loads
                nc.sync.dma_start(
                    stile[0:p_lo, :],
                    AP(
                        xtile[:].tensor,
                        xtile[:].offset,
                        [[HC * (FW + 1), p_lo], [1, HC * D]],
                    ),
                )
                x2 = s_pool.tile([cnt, l2], SDT, tag="X2")
                nc.sync.dma_start(x2[:], xtile[p_lo:m, p_lo * HC : p_lo * HC + l2])
                nc.sync.dma_start(
                    stile[p_lo:m, :],
                    AP(x2[:].tensor, x2[:].offset, [[l2 + HC, cnt], [1, HC * D]]),
                )

                # transposes: 96-col slices (3 dd x 32 hh) -> p = dd_l*32 + hh
                for tq in range(4):
                    tt = t_pool.tile([3 * HC, 4 * m], SDT, tag="T")
                    for s in range(4):
                        a = 4 * tq + s
                        nc.tensor.transpose(
                            tt[:, s * m : (s + 1) * m],
                            stile[:, a * 96 : (a + 1) * 96],
                            ident[:m, :m],
                        )
                    # O[p, a*Wr + (wb + i) - wboff] <- T[p, (a-4*tq)*m + i]
                    if wb < 256:
                        ot, Wr, wcol = otileA, 256, wb
                    else:
                        ot, Wr, wcol = otileB, 64, 0
                    dst = AP(
                        ot[:].tensor,
                        ot[:].offset + 4 * tq * Wr + wcol,
                        [[16 * Wr, 3 * HC], [Wr, 4], [1, m]],
                    )
                    if copy_parity & 1:
                        nc.scalar.copy(dst, tt[:].rearrange("p (a i) -> p a i", i=m))
                    else:
                        nc.vector.tensor_copy(dst, tt[:].rearrange("p (a i) -> p a i", i=m))
                    copy_parity += 1

                # out[47-(3a+dl), h0+hh, w] <- O[dl*32+hh, a*Wr+w'], dims
                # (hh, a, w); a-stride negative (dim1). The w<256 region
                # issues as soon as wb=128's copies land so only the small
                # w>=256 slice trails the last wblock. Issued from SP (sync
                # ring) to keep HWDGE issue time off the busy ACT engine.
                if wb == 128:
                    for dl in range(3):
                        dst = AP(
                            out_d,
                            (PAD - dl) * hw + h0 * W,
                            [[W, HC], [-3 * hw, 16], [1, 256]],
                        )
                        src = otileA[dl * HC : (dl + 1) * HC, :].rearrange(
                            "p (a w) -> p a w", w=256
                        )
                        nc.sync.dma_start(dst, src)
                elif wb == 256:
                    for dl in range(3):
                        dst = AP(
                            out_d,
                            (PAD - dl) * hw + h0 * W + 256,
                            [[W, HC], [-3 * hw, 16], [1, 64]],
                        )
                        src = otileB[dl * HC : (dl + 1) * HC, :].rearrange(
                            "p (a w) -> p a w", w=64
                        )
                        nc.sync.dma_start(dst, src)

    nc.compile()
    return nc


def kernel(left_feature: np.ndarray, right_feature: np.ndarray) -> np.ndarray:
    from concourse import bass_utils

    nc = build_nc()
    ident = np.eye(128, dtype=np.float32)
    in_maps = [
        {
            "left": np.ascontiguousarray(left_feature[b]),
            "right": np.ascontiguousarray(right_feature[b]),
            "ident": ident,
        }
        for b in range(B)
    ]
    res = bass_utils.run_bass_kernel_spmd(nc, in_maps, list(range(B)))
    return np.stack([res.results[b]["out"] for b in range(B)], axis=0)

